# revision 1
# baseline (speedup 1.0000x reference)
"""Causal self-attention on 8 Trainium2 cores.

Sharding: core c handles batch b = c // 2 and head group g = c % 2
(8 of 16 heads). Wqkv is split column-wise by head (tensor parallel),
Wproj row-wise; the host sums the two partial outputs per batch
(the all-reduce step).

Self-contained: hardcodes B=4, L=2048, D=1024, H=16.
"""

import numpy as np

import concourse.bass as bass  # noqa: F401  (bass types via bacc/tile)
import concourse.mybir as mybir
import concourse.tile as tile
from concourse import bacc
from concourse.bass_utils import run_bass_kernel_spmd

B, L, D, H, HD = 4, 2048, 1024, 16, 64
N_CORES = 8
HPC = 8            # heads per core
DG = HPC * HD      # 512: feature columns per head group
KT = D // 128      # 8 contraction tiles for the input projections
CIW = 1024         # query-chunk width in the attention phase
PVLAG = 5          # j-tiles of lag between the QK/exp stream and PV

f32 = mybir.dt.float32
f32r = mybir.dt.float32r


def _emit(nc, tc, xT, wqkv, wproj, trimask, ones8, out):
    FT = DG // 128  # 4 partition tiles of yT / wproj contraction
    with tc.tile_pool(name="persist", bufs=1) as persist:
        # q,k in transposed layout: rows = feature (q: 0-511, k: 512-1023),
        # cols = token. 8 partition tiles of [128, L].
        qkT = [persist.tile([128, L], f32r, tag=f"qkT{nt}", name=f"qkT{nt}") for nt in range(8)]
        # v in natural layout [token, head, hd+1]; last col = 1.0 so the PV
        # matmul also produces the softmax denominator (row 64 of its psum).
        vones = [
            persist.tile([128, HPC, HD + 1], f32r, tag=f"vo{mt}", name=f"vo{mt}")
            for mt in range(L // 128)
        ]
        tri = persist.tile([128, 128], f32r, tag="tri")
        nc.sync.dma_start(out=tri[:], in_=trimask[:, :].bitcast(f32r))

        # ---- phase 1: input projections ----
        with (
            tc.tile_pool(name="wq", bufs=1) as wpool,
            tc.tile_pool(name="xt", bufs=2) as xtpool,
            tc.tile_pool(name="ps1", bufs=6, space="PSUM") as ps1,
        ):
            w_sb = []
            for kt in range(KT):
                w_t = wpool.tile([128, 3 * DG], f32r, tag=f"w{kt}", name=f"w{kt}")
                nc.sync.dma_start(
                    out=w_t[:], in_=wqkv[kt * 128:(kt + 1) * 128, :].bitcast(f32r)
                )
                w_sb.append(w_t)
            for mc in range(L // 512):
                xts = []
                for kt in range(KT):
                    xt_t = xtpool.tile([128, 512], f32r, tag=f"xt{kt}", name=f"xt{kt}")
                    nc.sync.dma_start(
                        out=xt_t[:],
                        in_=xT[
                            kt * 128:(kt + 1) * 128, mc * 512:(mc + 1) * 512
                        ].bitcast(f32r),
                    )
                    xts.append(xt_t)
                # q,k → transposed layout
                for nt in range(8):
                    ps = ps1.tile([128, 512], f32, tag="ps1", name="ps1t")
                    for kt in range(KT):
                        nc.tensor.matmul(
                            ps[:],
                            w_sb[kt][:, nt * 128:(nt + 1) * 128],
                            xts[kt][:],
                            start=(kt == 0),
                            stop=(kt == KT - 1),
                        )
                    nc.vector.tensor_copy(
                        qkT[nt][:, mc * 512:(mc + 1) * 512], ps[:]
                    )
                # v → natural layout, interleaved with the ones column
                for mi in range(4):
                    mt = mc * 4 + mi
                    ps = ps1.tile([128, 512], f32, tag="ps1", name="ps1t")
                    for kt in range(KT):
                        nc.tensor.matmul(
                            ps[:],
                            xts[kt][:, mi * 128:(mi + 1) * 128],
                            w_sb[kt][:, 2 * DG:3 * DG],
                            start=(kt == 0),
                            stop=(kt == KT - 1),
                        )
                    nc.vector.tensor_copy(
                        vones[mt][:, :, 0:HD],
                        ps[:].rearrange("p (h d) -> p h d", d=HD),
                    )
                    nc.sync.dma_start(
                        out=vones[mt][:, :, HD], in_=ones8[:, :].bitcast(f32r)
                    )

        # ---- phases 2+3 ----
        with tc.tile_pool(name="ph23", bufs=1) as ph23:
            yT = [
                ph23.tile([128, L], f32r, tag=f"yT{ft}", name=f"yT{ft}")
                for ft in range(FT)
            ]
            _phase2(nc, tc, qkT, vones, tri, yT)
            _phase3(nc, tc, yT, wproj, out)


def _phase2(nc, tc, qkT, vones, tri, yT):
        # attention (S^T layout: rows=key j, cols=query i)
        with (
            tc.tile_pool(name="pp", bufs=8) as ppool,
            tc.tile_pool(name="rr", bufs=2) as rpool,
            tc.tile_pool(name="pss", bufs=2, space="PSUM") as ps_s,
            tc.tile_pool(name="psy", bufs=2, space="PSUM") as ps_y,
        ):
            for hh in range(HPC):
                q_t, q_off = qkT[hh // 2], 64 * (hh % 2)
                k_t, k_off = qkT[4 + hh // 2], 64 * (hh % 2)
                for ci in range(L // CIW):
                    njt = (ci + 1) * CIW // 128
                    yp = ps_y.tile([65, CIW], f32, tag="yp", name="ypt")
                    ptiles = [None] * njt
                    offs = [128 * jt - CIW * ci for jt in range(njt)]

                    def emit_qk(jt):
                        off = offs[jt]
                        sp = ps_s.tile([128, CIW], f32, tag="sp", name="spt")
                        for lo in range(0, CIW, 512):
                            if off >= lo + 512:
                                continue  # fully masked column range
                            nc.tensor.matmul(
                                sp[:, lo:lo + 512],
                                k_t[k_off:k_off + 64, jt * 128:(jt + 1) * 128],
                                q_t[
                                    q_off:q_off + 64,
                                    ci * CIW + lo: ci * CIW + lo + 512,
                                ],
                                start=True,
                                stop=True,
                            )
                        pt = ppool.tile([128, CIW], f32r, tag="pt", name="ptt")
                        a = max(off, 0)
                        if off > 0 and off % 512:
                            nc.gpsimd.memset(
                                pt[:, (off // 512) * 512: off].bitcast(f32), 0.0
                            )
                        nc.scalar.activation(
                            pt[:, a:CIW],
                            sp[:, a:CIW],
                            mybir.ActivationFunctionType.Exp,
                            scale=float(1.0 / np.sqrt(HD)),
                        )
                        if off >= 0:
                            nc.vector.tensor_mul(
                                pt[:, off:off + 128], pt[:, off:off + 128], tri[:]
                            )
                        ptiles[jt] = pt

                    def emit_pv(jt):
                        off = offs[jt]
                        for lo in range(0, CIW, 512):
                            if off >= lo + 512:
                                continue
                            # last j-tile contributing to this column range
                            last = min(njt - 1, (CIW * ci + lo + 511) // 128)
                            nc.tensor.matmul(
                                yp[:, lo:lo + 512],
                                vones[jt][:, hh, :],
                                ptiles[jt][:, lo:lo + 512],
                                start=(jt == 0),
                                stop=(jt == last),
                            )

                    for jt in range(njt + PVLAG):
                        if jt < njt:
                            emit_qk(jt)
                        if jt - PVLAG >= 0:
                            emit_pv(jt - PVLAG)

                    # normalize: yT = y' / rowsum (rowsum lives in row 64).
                    # 1/r computed as exp(-ln r) on the scalar engine: the
                    # DVE reciprocal is an 8-pass iterative op (6.5us/call)
                    rln = rpool.tile([1, CIW], f32, tag="rln", name="rlnt")
                    nc.scalar.activation(
                        rln[:], yp[64:65, :], mybir.ActivationFunctionType.Ln
                    )
                    rrec = rpool.tile([1, CIW], f32, tag="rrec", name="rrect")
                    nc.scalar.activation(
                        rrec[:], rln[:], mybir.ActivationFunctionType.Exp,
                        scale=-1.0,
                    )
                    rrecb = rpool.tile([64, CIW], f32, tag="rrecb", name="rrecbt")
                    nc.gpsimd.partition_broadcast(rrecb[:], rrec[:])
                    nc.vector.tensor_mul(
                        yT[hh // 2][
                            64 * (hh % 2): 64 * (hh % 2) + 64,
                            ci * CIW:(ci + 1) * CIW,
                        ],
                        yp[0:64, :],
                        rrecb[:],
                    )


def _phase3(nc, tc, yT, wproj, out):
        # output projection (partial: host adds the two head groups)
        FT = DG // 128
        with (
            tc.tile_pool(name="wp", bufs=1) as wppool,
            tc.tile_pool(name="ob", bufs=3) as opool,
            tc.tile_pool(name="ps3", bufs=4, space="PSUM") as ps3,
        ):
            wp_sb = []
            for ft in range(FT):
                wp_t = wppool.tile([128, D], f32r, tag=f"wp{ft}", name=f"wp{ft}")
                nc.sync.dma_start(
                    out=wp_t[:], in_=wproj[ft * 128:(ft + 1) * 128, :].bitcast(f32r)
                )
                wp_sb.append(wp_t)
            for it in range(L // 128):
                o_t = opool.tile([128, D], f32, tag="ot", name="ott")
                for nc_ in range(D // 512):
                    ps = ps3.tile([128, 512], f32, tag="ps3", name="ps3t")
                    for ft in range(FT):
                        nc.tensor.matmul(
                            ps[:],
                            yT[ft][:, it * 128:(it + 1) * 128],
                            wp_sb[ft][:, nc_ * 512:(nc_ + 1) * 512],
                            start=(ft == 0),
                            stop=(ft == FT - 1),
                        )
                    nc.vector.tensor_copy(o_t[:, nc_ * 512:(nc_ + 1) * 512], ps[:])
                nc.sync.dma_start(
                    out=out[it * 128:(it + 1) * 128, :], in_=o_t[:]
                )


def build():
    nc = bacc.Bacc(
        "TRN2", target_bir_lowering=False, debug=False, num_devices=N_CORES
    )
    xT = nc.dram_tensor("xT", [D, L], f32, kind="ExternalInput").ap()
    wqkv = nc.dram_tensor("wqkv", [D, 3 * DG], f32, kind="ExternalInput").ap()
    wproj = nc.dram_tensor("wproj", [DG, D], f32, kind="ExternalInput").ap()
    trimask = nc.dram_tensor("trimask", [128, 128], f32, kind="ExternalInput").ap()
    ones8 = nc.dram_tensor("ones8", [128, HPC], f32, kind="ExternalInput").ap()
    out = nc.dram_tensor("out", [L, D], f32, kind="ExternalOutput").ap()
    with tile.TileContext(nc) as tc:
        _emit(nc, tc, xT, wqkv, wproj, trimask, ones8, out)
    nc.compile()
    return nc


def shard_inputs(x, Wqkv, Wproj):
    tri = np.triu(np.ones((128, 128), dtype=np.float32))
    in_maps = []
    for c in range(N_CORES):
        b, g = c // 2, c % 2
        wqkv_c = np.concatenate(
            [
                Wqkv[:, DG * g:DG * (g + 1)],
                Wqkv[:, D + DG * g:D + DG * (g + 1)],
                Wqkv[:, 2 * D + DG * g:2 * D + DG * (g + 1)],
            ],
            axis=1,
        )
        in_maps.append(
            {
                "xT": np.ascontiguousarray(x[b].T),
                "wqkv": np.ascontiguousarray(wqkv_c),
                "wproj": np.ascontiguousarray(Wproj[DG * g:DG * (g + 1), :]),
                "trimask": tri,
                "ones8": np.ones((128, HPC), dtype=np.float32),
            }
        )
    return in_maps


_NC_CACHE = {}


def get_nc():
    if "nc" not in _NC_CACHE:
        _NC_CACHE["nc"] = build()
    return _NC_CACHE["nc"]


def run_sharded(in_maps, **kwargs):
    return run_bass_kernel_spmd(
        get_nc(), in_maps, core_ids=list(range(N_CORES)), **kwargs
    )


def kernel(x, Wqkv, Wproj, attn_mask, key_padding_mask):
    # attn_mask is causal and key_padding_mask is all-False for this
    # problem; both are hardcoded into the device program.
    x = np.asarray(x, dtype=np.float32)
    in_maps = shard_inputs(
        x, np.asarray(Wqkv, dtype=np.float32), np.asarray(Wproj, dtype=np.float32)
    )
    res = run_sharded(in_maps)
    out = np.empty((B, L, D), dtype=np.float32)
    for b in range(B):
        out[b] = res.results[2 * b]["out"] + res.results[2 * b + 1]["out"]
    return out

